# revision 21
# baseline (speedup 1.0000x reference)
"""Multi-head low-rank-score (LSR) causal attention on 8 trn2 NeuronCores.

Sharding: core = 4*b + g owns batch b and heads [4g, 4g+4).

Key structure (v2 — HAM-warm redesign):
- Q/K projections are never materialized: q_lr^T = (Wq[:,cols] @ Wq_lsr)^T
  @ x^T, folded on the host into one [D,128] effective weight per side.
- Softmax max-stats pass: S in q-layout -> DVE row-max (fused diagonal
  mask via tensor_tensor_reduce) -> GpSimd max-tree -> PE transpose ->
  ScalarE negate-evac -> GpSimd scatter into the augmented -m rows.
  No DMA engine involvement.
- S^T pass: per (l-group, j): two quadrant matmuls into a 2-bank PSUM
  tile, one merged exp() on ScalarE (causally clipped via 3D APs),
  AV accumulates in 4 PSUM banks with the denominators coming from a
  leading ones-column in V.
- PSUM: 2x [128,2,512] S^T group tiles + 4x [65,512] AV accumulators
  = 8 banks; stats / o_proj / V-proj matmuls reuse the S^T slots in
  bursts at chunk boundaries so the PE queue never idles long enough
  for HAM to re-throttle the clock.
- Engine balance: ScalarE = exp + half the evacs; DVE = stats reduces,
  V/ctx evacs, fast-approx reciprocal; GpSimd = SBUF-side small ops.
"""

import numpy as np
import ml_dtypes

B = 2
T = 2048
D = 1024
H = 16
DH = 64
R = 32
HPC = 4  # heads per core
OC = HPC * DH  # 256 V-cols per core
NCORES = 8
SCALE = 1.0 / float(np.sqrt(np.float32(R)))
NEG = -30000.0
NT = T // 128  # 16 key/query tiles
NCH = T // 512  # 4 query chunks

_cache = {}


def _build():
    import concourse.bacc as bacc
    import concourse.mybir as mybir
    from concourse.tile import TileContext

    F32 = mybir.dt.float32
    F32R = mybir.dt.float32r
    BF16 = mybir.dt.bfloat16
    EXP = mybir.ActivationFunctionType.Exp
    COPY = mybir.ActivationFunctionType.Copy
    IDENT = mybir.ActivationFunctionType.Identity
    MAX = mybir.AluOpType.max
    ADD = mybir.AluOpType.add
    AXX = mybir.AxisListType.X

    nc = bacc.Bacc("TRN2", target_bir_lowering=False, debug=False,
                   num_devices=NCORES)

    xT = nc.declare_dram_parameter("xT", [D, T], F32R, isOutput=False)
    wv = nc.declare_dram_parameter("wv", [D, OC], F32R, isOutput=False)
    wqle = nc.declare_dram_parameter("wqle", [D, 128], F32R, isOutput=False)
    wkle = nc.declare_dram_parameter("wkle", [D, 128], F32R, isOutput=False)
    blq = nc.declare_dram_parameter("blq", [128, 1], F32, isOutput=False)
    blk = nc.declare_dram_parameter("blk", [128, 1], F32, isOutput=False)
    wo = nc.declare_dram_parameter("wo", [OC, D], BF16, isOutput=False)
    # [16, T] row j': NEG where t < 128*j' else 0 (q-side causal aug rows)
    indq = nc.declare_dram_parameter("indq", [NT, T], F32R, isOutput=False)
    # [17, T]: row 0 = ones; rows 1+j': 1.0 on k-tile j' cols else 0
    okq = nc.declare_dram_parameter("okq", [NT + 1, T], F32R, isOutput=False)
    # [128, 1024]: zeros, last 128 cols = triu(NEG, 1) (stats diag mask)
    tpad = nc.declare_dram_parameter("tpad", [128, 1024], F32, isOutput=False)
    # [128, 2, 128]: tril(NEG, -1) twice (S^T diag mask per group)
    trid2 = nc.declare_dram_parameter("trid2", [128, 2, 128], F32,
                                      isOutput=False)
    # [128, 2, 128]: triu(NEG, 1) twice (stats diag mask per group pair)
    triq2 = nc.declare_dram_parameter("triq2", [128, 2, 128], F32,
                                      isOutput=False)
    sel2 = nc.declare_dram_parameter("sel2", [2, 128], F32R, isOutput=False)
    idf = nc.declare_dram_parameter("idf", [128, 128], F32, isOutput=False)
    yT = nc.declare_dram_parameter("yT", [D, T], F32, isOutput=True)

    with TileContext(nc) as tc:
        with (
            nc.allow_low_precision(reason="f32r scores / bf16 P,V / approx recip"),
            tc.tile_pool(name="persist", bufs=1) as pp,
            tc.tile_pool(name="ps", bufs=1, space="PSUM") as ps,
            tc.tile_pool(name="work", bufs=2) as wk,
        ):
            # ---- persistent SBUF tiles
            wv_t = [pp.tile([128, OC], F32R, tag=f"wv{i}", name=f"wv{i}")
                    for i in range(8)]
            wqle_t = [pp.tile([128, 128], F32R, tag=f"wqle{i}", name=f"wqle{i}")
                      for i in range(8)]
            wkle_t = [pp.tile([128, 128], F32R, tag=f"wkle{i}", name=f"wkle{i}")
                      for i in range(8)]
            blq_t = pp.tile([128, 1], F32, tag="blq")
            blk_t = pp.tile([128, 1], F32, tag="blk")
            wo_t = [pp.tile([128, D], BF16, tag=f"wo{p}", name=f"wo{p}")
                    for p in range(2)]
            tpad_t = pp.tile([128, 1024], F32, tag="tpad")
            trid2_t = pp.tile([128, 2, 128], F32, tag="trid2")
            triq2_t = pp.tile([128, 2, 128], F32, tag="triq2")
            sel2_t = pp.tile([2, 128], F32R, tag="sel2")
            idf_t = pp.tile([128, 128], F32, tag="idf")
            # augmented lr tiles, one per head pair p (heads 2p, 2p+1)
            # rows [64l, 64l+32): q_lr^T/k_lr^T of head 2p+l (q side scaled)
            # row 64l+32: -m (q side) / ones (k side)
            # rows [64l+33, 64l+49): indq (q side) / k-tile selectors (k side)
            qaug = [pp.tile([128, T], F32R, tag=f"qaug{p}", name=f"qaug{p}")
                    for p in range(2)]
            kaug = [pp.tile([128, T], F32R, tag=f"kaug{p}", name=f"kaug{p}")
                    for p in range(2)]
            # V with leading ones column per head: [128, h, 0] = 1,
            # [128, h, 1:65] = V_h rows for this k-tile
            vaug = [pp.tile([128, HPC, DH + 1], BF16, tag=f"va{j}",
                            name=f"va{j}") for j in range(NT)]
            ctxr = [[pp.tile([128, 512], BF16, tag=f"cx{p}_{c}",
                             name=f"cx{p}_{c}") for c in range(NCH)]
                    for p in range(2)]
            xt_t = [pp.tile([128, T], F32R, tag=f"x{i}", name=f"x{i}")
                    for i in range(8)]

            # ---- input DMAs (ordered: lr weights -> x -> consts -> V/o)
            for i in range(8):
                nc.sync.dma_start(out=wqle_t[i][:],
                                  in_=wqle[128 * i:128 * i + 128, :])
                nc.sync.dma_start(out=wkle_t[i][:],
                                  in_=wkle[128 * i:128 * i + 128, :])
            nc.sync.dma_start(out=blq_t[:], in_=blq[:])
            nc.sync.dma_start(out=blk_t[:], in_=blk[:])
            for p in range(2):
                nc.sync.dma_start(out=wo_t[p][:],
                                  in_=wo[128 * p:128 * p + 128, :])
            for ch in range(NCH):
                for i in range(8):
                    nc.sync.dma_start(
                        out=xt_t[i][:, 512 * ch:512 * ch + 512],
                        in_=xT[128 * i:128 * i + 128,
                               512 * ch:512 * ch + 512])
            nc.sync.dma_start(out=tpad_t[:], in_=tpad[:])
            nc.sync.dma_start(out=trid2_t[:], in_=trid2[:])
            nc.sync.dma_start(out=triq2_t[:], in_=triq2[:])
            nc.sync.dma_start(out=sel2_t[:], in_=sel2[:])
            nc.sync.dma_start(out=idf_t[:], in_=idf[:])
            for p in range(2):
                for l in range(2):
                    nc.sync.dma_start(
                        out=qaug[p][64 * l + 33:64 * l + 49, :], in_=indq[:])
                    nc.sync.dma_start(
                        out=kaug[p][64 * l + 32:64 * l + 49, :], in_=okq[:])
            for i in range(8):
                nc.sync.dma_start(out=wv_t[i][:],
                                  in_=wv[128 * i:128 * i + 128, :])
            # ones columns of vaug (constant, written once)
            for j in range(NT):
                nc.gpsimd.memset(vaug[j][:, :, DH:DH + 1], 1.0)

            # ---- PSUM slot machinery: 2 group tags (2 banks each) + 4 AV
            # AV banks hold the [0:65] accumulators; partitions [66:128) are
            # space [96:128) is the target for PE-warming dummy matmuls (HAM
            # re-throttles the PE clock to 1.2 GHz after any idle window, so
            # the PE queue must never drain).
            av_t = [ps.tile([128, 512], F32, tag=f"a{u % 2}{u // 2}",
                            name=f"a{u % 2}{u // 2}") for u in range(4)]
            dummy_ctr = [0]

            def dummy(n=1, cols=512):
                for _ in range(n):
                    u = dummy_ctr[0] % 4
                    dummy_ctr[0] += 1
                    nc.tensor.matmul(
                        av_t[u][96:128, 0:cols], wo_t[0][:, 0:32],
                        wo_t[1][:, 0:cols], start=True, stop=True,
                        tile_position=(0, 96), skip_group_check=True)

            slot_ctr = [0]

            def gslot():
                """Next [128, 512] PSUM scratch view, round-robin over the
                4 half-group slots."""
                i = slot_ctr[0] % 4
                slot_ctr[0] += 1
                g = ps.tile([128, 2, 512], F32, tag=f"g{i // 2}",
                            name=f"g{i // 2}")
                return g[:, i % 2, :]

            # ---- phase P: lr projections + V projection
            aug = (qaug, kaug)
            wle = (wqle_t, wkle_t)
            bl = (blq_t, blk_t)
            dummy(60)  # bridge the input-DMA window, warm the PE early
            for ch in range(NCH):
                for side in range(2):
                    v = gslot()
                    for kk in range(8):
                        nc.tensor.matmul(
                            v, wle[side][kk][:],
                            xt_t[kk][:, 512 * ch:512 * ch + 512],
                            start=(kk == 0), stop=(kk == 7))
                    # 4 partition-block evacs -> aug lr rows (+ lr bias)
                    for u in range(4):
                        p, l = u % 2, u // 2
                        dst = aug[side][p][64 * l:64 * l + 32,
                                           512 * ch:512 * ch + 512]
                        src = v[32 * u:32 * u + 32, :]
                        bias = bl[side][32 * u:32 * u + 32, :]
                        if u < 2:
                            nc.scalar.activation(dst, src, IDENT, bias=bias,
                                                 scale=1.0)
                        else:
                            nc.vector.tensor_scalar_add(dst, src, bias)
            for tt in range(NT):
                v = gslot()
                for kk in range(8):
                    nc.tensor.matmul(
                        v[:, 0:OC], xt_t[kk][:, 128 * tt:128 * tt + 128],
                        wv_t[kk][:], start=(kk == 0), stop=(kk == 7))
                nc.vector.tensor_copy(
                    vaug[tt][:, :, 0:DH],
                    v[:, 0:OC].rearrange("p (h d) -> p h d", h=HPC))

            # ---- stats helpers -------------------------------------
            # Per (i, cc, l): matmul the two p-streams of group l into a
            # 2-bank PSUM pair and row-max them in one merged DVE reduce.
            # Per i (after its last chunk): 2nd-level max -> PE transpose ->
            # ScalarE negate-evac -> 4 tiny DMAs into the -m rows of qaug.
            mx_tiles = {}

            def stats_mms(i, cc, l, g):
                """Emit the stats matmuls for (i, cc, stream group l) into
                psum group tile g; returns nothing. g[:, p, :] is used."""
                nch = i // 4 + 1
                ncols = min(512, 128 * (i + 1) - 512 * cc)
                for p in range(2):
                    nc.tensor.matmul(
                        g[:, p, 0:ncols],
                        qaug[p][64 * l:64 * l + R, 128 * i:128 * i + 128],
                        kaug[p][64 * l:64 * l + R, 512 * cc:512 * cc + ncols],
                        start=True, stop=True, tile_position=(64 * l, 0))
                if cc == nch - 1:
                    nc.vector.tensor_add(
                        g[:, :, ncols - 128:ncols],
                        g[:, :, ncols - 128:ncols], triq2_t[:])
                mxt = mx_tiles[i]
                nc.vector.tensor_reduce(
                    mxt[:, 4 * cc + 2 * l:4 * cc + 2 * l + 2],
                    g[:, :, 0:ncols], axis=AXX, op=MAX)

            def stats_tail(i, pv):
                """2nd-level reduce + transpose (into psum region pv
                [0:4, 0:128]) + negate-evac + -m row scatter for tile i."""
                nch = i // 4 + 1
                mxt = mx_tiles.pop(i)
                t4 = wk.tile([128, 4], F32, tag="t4", name="t4")
                nc.vector.tensor_reduce(
                    t4[:],
                    mxt[:, 0:4 * nch].rearrange("p (c u) -> p u c", u=4),
                    axis=AXX, op=MAX)
                nc.tensor.transpose(pv[0:4, 0:128], t4[:], idf_t[:])
                ns = wk.tile([4, 128], F32R, tag="ns", name="ns")
                nc.scalar.activation(ns[:], pv[0:4, 0:128], COPY,
                                     scale=-1.0)
                for u in range(4):
                    p, l = u % 2, u // 2
                    nc.sync.dma_start(
                        out=qaug[p][64 * l + 32:64 * l + 33,
                                    128 * i:128 * i + 128],
                        in_=ns[u:u + 1, :])

            def emit_stats_burst(c):
                """Upfront (pre-T) stats for chunk c, gslot-rotated, with
                PE-warming dummies."""
                for i in range(4 * c, 4 * c + 4):
                    nch = i // 4 + 1
                    mx_tiles[i] = wk.tile([128, 16], F32, tag="mx",
                                          name="mx")
                    for cc in range(nch):
                        for l in range(2):
                            g = ps.tile([128, 2, 512], F32, tag=f"g{l}",
                                        name=f"g{l}")
                            stats_mms(i, cc, l, g)
                    stats_tail(i, gslot())

            # seam work queue: consumed two sub-slots (one group) at a time
            # inside the T(c) j-loops, right after that group's exp.
            # entries: ("stats", i, cc, l) | ("stat_tail", i) |
            #          ("oproj", c, ot, sub) | ("dummy", n)
            seam_q = []

            def fill_stats_seams(c):
                for i in range(4 * c, 4 * c + 4):
                    nch = i // 4 + 1
                    for cc in range(nch):
                        seam_q.append(("stats", i, cc, 0))
                        seam_q.append(("stats", i, cc, 1))
                    seam_q.append(("stat_tail", i))

            def fill_oproj_seams(c):
                for ot in range(0, 8, 2):
                    seam_q.append(("oproj", c, ot))

            def consume_seam(l, g):
                """Emit one seam unit into just-freed group tile g (group l).
                Stats units must match l; rotate queue if misaligned."""
                for k in range(len(seam_q)):
                    kind = seam_q[k][0]
                    if kind == "stats" and seam_q[k][3] != l:
                        continue
                    unit = seam_q.pop(k)
                    break
                else:
                    return
                if unit[0] == "stats":
                    _, i, cc, _ = unit
                    if i not in mx_tiles:
                        mx_tiles[i] = wk.tile([128, 16], F32, tag="mx",
                                              name="mx")
                    stats_mms(i, cc, l, g)
                elif unit[0] == "stat_tail":
                    stats_tail(unit[1], g[:, 0, :])
                elif unit[0] == "oproj":
                    _, c_, ot = unit
                    for sub in range(2):
                        o = ot + (0 if sub == 0 else 1)
                        y = g[:, sub, :]
                        for p in range(2):
                            nc.tensor.matmul(
                                y, wo_t[p][:, 128 * o:128 * o + 128],
                                ctxr[p][c_][:], start=(p == 0),
                                stop=(p == 1))
                        ysb = wk.tile([128, 512], F32, tag="ysb",
                                      name="ysb")
                        if o % 2 == 0:
                            nc.scalar.copy(ysb[:], y)
                        else:
                            nc.vector.tensor_copy(ysb[:], y)
                        nc.sync.dma_start(
                            out=yT[128 * o:128 * o + 128,
                                   512 * c_:512 * c_ + 512],
                            in_=ysb[:])

            def flush_stats_before(c):
                """Emit any still-queued stats units for chunk c as a
                burst (they must precede T(c)'s reads of the -m rows)."""
                rest = []
                for unit in seam_q:
                    tgt = unit[1] // 4
                    if unit[0] in ("stats", "stat_tail") and tgt == c:
                        if unit[0] == "stats":
                            _, i, cc, l = unit
                            if i not in mx_tiles:
                                mx_tiles[i] = wk.tile([128, 16], F32,
                                                      tag="mx", name="mx")
                            g = ps.tile([128, 2, 512], F32, tag=f"g{l}",
                                        name=f"g{l}")
                            stats_mms(i, cc, l, g)
                        else:
                            stats_tail(unit[1], gslot())
                    else:
                        rest.append(unit)
                seam_q[:] = rest

            # ---- T(c): S^T + exp + AV with seam-interleaved stats/o_proj
            def emit_T(c):
                njt = 4 * c + 4
                av = {(u % 2, u // 2): av_t[u][0:DH + 1, :] for u in range(4)}
                for j in range(njt):
                    dd = j - 4 * c
                    am = (0, 128, 256, 256)[dd] if dd >= 0 else 0
                    pts = []
                    for l in range(2):
                        g = ps.tile([128, 2, 512], F32, tag=f"g{l}",
                                    name=f"g{l}")
                        for p in range(2):
                            nc.tensor.matmul(
                                g[:, p, am:512],
                                kaug[p][64 * l:64 * l + R + 17,
                                        128 * j:128 * j + 128],
                                qaug[p][64 * l:64 * l + R + 17,
                                        512 * c + am:512 * c + 512],
                                start=True, stop=True,
                                tile_position=(64 * l, 0))
                        if dd >= 0:
                            d0 = 128 * dd
                            nc.vector.tensor_add(g[:, :, d0:d0 + 128],
                                                 g[:, :, d0:d0 + 128],
                                                 trid2_t[:])
                        pt = wk.tile([128, 2, 512], BF16, tag=f"pt{l}",
                                     name=f"pt{l}")
                        nc.scalar.activation(pt[:, :, am:512],
                                             g[:, :, am:512], EXP)
                        pts.append(pt)
                        # seam: this group's banks are free until the next
                        # S^T write; slip in stats/o_proj matmuls
                        consume_seam(l, g)
                    for l in range(2):
                        for p in range(2):
                            h = 2 * p + l
                            nc.tensor.matmul(
                                av[(p, l)][:, am:512], vaug[j][:, h, :],
                                pts[l][:, p, am:512],
                                start=(j == 0), stop=(j == njt - 1))
                # epilogue: denominator rows -> broadcast -> 1/x -> ctx
                for p in range(2):
                    rd = wk.tile([2, 512], F32, tag=f"rd{p}", name=f"rd{p}")
                    for l in range(2):
                        dl = wk.tile([1, 512], F32, tag=f"dl{l}",
                                     name=f"dl{l}")
                        nc.scalar.copy(dl[:], av[(p, l)][DH:DH + 1, :])
                        nc.sync.dma_start(out=rd[l:l + 1, :], in_=dl[:])
                    bv_ = gslot()
                    nc.tensor.matmul(bv_, sel2_t[:], rd[:].bitcast(F32R),
                                     start=True, stop=True)
                    rv = wk.tile([128, 512], F32, tag="rv", name="rv")
                    nc.vector.reciprocal_approx_fast(out=rv[:], in_=bv_)
                    cf = wk.tile([128, 512], F32, tag="cf", name="cf")
                    for l in range(2):
                        nc.vector.tensor_copy(cf[64 * l:64 * l + 64, :],
                                              av[(p, l)][0:DH, :])
                    nc.vector.tensor_mul(ctxr[p][c][:], cf[:], rv[:])

            def emit_oproj_burst(c):
                for ot in range(8):
                    y = gslot()
                    for p in range(2):
                        nc.tensor.matmul(
                            y, wo_t[p][:, 128 * ot:128 * ot + 128],
                            ctxr[p][c][:], start=(p == 0), stop=(p == 1))
                    ysb = wk.tile([128, 512], F32, tag="ysb", name="ysb")
                    if ot % 2 == 0:
                        nc.scalar.copy(ysb[:], y)
                    else:
                        nc.vector.tensor_copy(ysb[:], y)
                    nc.sync.dma_start(
                        out=yT[128 * ot:128 * ot + 128,
                               512 * c:512 * c + 512],
                        in_=ysb[:])

            emit_stats_burst(0)
            emit_stats_burst(1)
            fill_stats_seams(2)
            emit_T(0)
            fill_stats_seams(3)
            fill_oproj_seams(0)
            flush_stats_before(2)
            emit_T(1)
            fill_oproj_seams(1)
            flush_stats_before(2)
            emit_T(2)
            fill_oproj_seams(2)
            flush_stats_before(3)
            emit_T(3)
            emit_oproj_burst(3)

    nc.compile()
    return nc


def _consts():
    indq = np.zeros((NT, T), np.float32)
    for j in range(NT):
        indq[j, :128 * j] = NEG
    okq = np.zeros((NT + 1, T), np.float32)
    okq[0] = 1.0
    for j in range(NT):
        okq[1 + j, 128 * j:128 * j + 128] = 1.0
    tpad = np.zeros((128, 1024), np.float32)
    tpad[:, 896:1024] = np.triu(np.full((128, 128), NEG, np.float32), 1)
    trik = np.tril(np.full((128, 128), NEG, np.float32), -1)
    trid2 = np.stack([trik, trik], axis=1)  # [128, 2, 128]
    triqm = np.triu(np.full((128, 128), NEG, np.float32), 1)
    triq2 = np.stack([triqm, triqm], axis=1)  # [128, 2, 128]
    sel2 = np.zeros((2, 128), np.float32)
    sel2[0, :64] = 1.0
    sel2[1, 64:] = 1.0
    idf = np.eye(128, dtype=np.float32)
    return indq, okq, tpad, trid2, triq2, sel2, idf


def kernel(x, Wq, bq, Wk, bk, Wv, bv, Wo, bo, Wq_lsr, Wk_lsr):
    from concourse.bass_utils import run_bass_kernel_spmd

    if "nc" not in _cache:
        _cache["nc"] = _build()
    nc = _cache["nc"]

    x = np.asarray(x, np.float32)
    Wq = np.asarray(Wq, np.float32)
    Wk = np.asarray(Wk, np.float32)
    Wv = np.asarray(Wv, np.float32)
    Wo = np.asarray(Wo, np.float32)
    bq = np.asarray(bq, np.float32)
    bk = np.asarray(bk, np.float32)
    bv = np.asarray(bv, np.float32)
    bo = np.asarray(bo, np.float32)
    Wq_lsr = np.asarray(Wq_lsr, np.float32)
    Wk_lsr = np.asarray(Wk_lsr, np.float32)

    indq, okq, tpad, trid2, triq2, sel2, idf = _consts()
    # local head order for the lr-psum partition blocks: u=(p,l) -> h=2p+l
    horder = [0, 2, 1, 3]
    in_maps = []
    for core in range(NCORES):
        b, g = divmod(core, 4)
        hs = HPC * g
        cols = slice(DH * hs, DH * hs + OC)
        # effective low-rank projection weights: [D, 4*R], block u = head
        # horder[u]; q side carries the 1/sqrt(R) score scale
        wqle_blocks, wkle_blocks, blq_v, blk_v = [], [], [], []
        for u in range(4):
            h = hs + horder[u]
            wq_h = Wq[:, DH * h:DH * h + DH] @ Wq_lsr[h] * SCALE
            wk_h = Wk[:, DH * h:DH * h + DH] @ Wk_lsr[h]
            wqle_blocks.append(wq_h)
            wkle_blocks.append(wk_h)
            blq_v.append(bq[DH * h:DH * h + DH] @ Wq_lsr[h] * SCALE)
            blk_v.append(bk[DH * h:DH * h + DH] @ Wk_lsr[h])
        in_maps.append({
            "xT": np.ascontiguousarray(x[b].T),
            "wv": np.ascontiguousarray(Wv[:, cols]),
            "wqle": np.ascontiguousarray(np.concatenate(wqle_blocks, axis=1)),
            "wkle": np.ascontiguousarray(np.concatenate(wkle_blocks, axis=1)),
            "blq": np.concatenate(blq_v).reshape(128, 1).astype(np.float32),
            "blk": np.concatenate(blk_v).reshape(128, 1).astype(np.float32),
            "wo": np.ascontiguousarray(Wo[cols, :]).astype(ml_dtypes.bfloat16),
            "indq": indq, "okq": okq, "tpad": tpad, "trid2": trid2,
            "triq2": triq2, "sel2": sel2, "idf": idf,
        })

    res = run_bass_kernel_spmd(nc, in_maps, list(range(NCORES)),
                               **_cache.get("run_kwargs", {}))
    _cache["last_results"] = res

    y = np.zeros((B, T, D), np.float32)
    for core in range(NCORES):
        b = core // 4
        y[b] += res.results[core]["yT"].T
    y += (bv @ Wo + bo)[None, None, :]
    return y
